# revision 21
# baseline (speedup 1.0000x reference)
import sys

sys.path.insert(0, "/opt/trn_rl_repo")
import numpy as np
import concourse.bass as bass
import concourse.mybir as mybir
import concourse.tile as tile
from concourse.bass_utils import run_bass_kernel_spmd

NQ = 14
LAYERS = 3
B = 256
NCORES = 8
BS = B // NCORES  # 32 samples per core
PI = float(np.pi)
TWO_PI = float(2 * np.pi)
SHIFT = float(15 * np.pi)
F32 = mybir.dt.float32
ALU = mybir.AluOpType
ACTF = mybir.ActivationFunctionType

# (layer, round, half) for the 10 on-device gate rounds; layer-0 round-0 is
# folded into the host-computed initial state.
R_STEPS = [
    (0, 1, "h"), (0, 1, "l"),
    (1, 0, "h"), (1, 0, "l"), (1, 1, "h"), (1, 1, "l"),
    (2, 0, "h"), (2, 0, "l"), (2, 1, "h"), (2, 1, "l"),
]


def _patch_drains(nc, max_waits=1):
    # walrus codegen rejects instructions carrying >1 sync waits; split
    # excess waits onto same-engine nop instructions inserted just before
    eng_objs = [nc.tensor, nc.vector, nc.scalar, nc.gpsimd, nc.sync]
    by_key = {}

    def fresh_nop(eo):
        nb = eo.nop().ins
        for bb2 in nc.main_func.blocks:
            il2 = bb2.instructions
            if il2 and il2[-1].name == nb.name:
                il2.pop()
                return nb
        raise AssertionError("fresh nop not found at any block tail")

    for eo in eng_objs:
        probe = fresh_nop(eo)
        by_key.setdefault(probe.engine, eo)

    for bb in nc.main_func.blocks:
        il = bb.instructions
        i = 0
        while i < len(il):
            ins = il[i]
            si = ins.sync_info
            if si is not None and si.on_wait and len(si.on_wait) > max_waits:
                waits = list(si.on_wait)
                si.on_wait = waits[:max_waits]
                eo = by_key[ins.engine]
                new_nops = []
                for w in waits[max_waits:]:
                    nb = fresh_nop(eo)
                    nsi = nb.sync_info
                    if nsi is None:
                        nb.sync_info = type(si)(on_wait=[w], on_update=[])
                    else:
                        nsi.on_wait = [w]
                    new_nops.append(nb)
                il[i:i] = new_nops
                i += len(new_nops) + 1
            else:
                i += 1


_PROG = None


def _build():
    global _PROG
    if _PROG is not None:
        return _PROG
    nc = bass.Bass()
    init_e = nc.declare_dram_parameter("init", [128, 256], F32, isOutput=False)
    rall_e = nc.declare_dram_parameter("rall", [128, 5120], F32, isOutput=False)
    # diag cos/sin rank-4 matmul data, packed into 4 partition groups
    # (base 32g): rows 4g+0..3 hold 24 samples each along the free dim
    gdd_e = nc.declare_dram_parameter("gdd", [16, 24 * 128], F32, isOutput=False)
    fdd_e = nc.declare_dram_parameter("fdd", [16, 24 * 256], F32, isOutput=False)
    qm_e = nc.declare_dram_parameter("qm", [128, 256], F32, isOutput=False)
    id_e = nc.declare_dram_parameter("ident", [128, 128], F32, isOutput=False)
    misc_e = nc.declare_dram_parameter("misc", [128, 2], F32, isOutput=False)
    y_e = nc.declare_dram_parameter("y", [1, 32], F32, isOutput=True)

    with tile.TileContext(nc) as tc:
        with tc.tile_pool(name="const", bufs=1) as cpool, \
             tc.tile_pool(name="state", bufs=1) as spool, \
             tc.tile_pool(name="scr", bufs=3) as scrp, \
             tc.tile_pool(name="tscr", bufs=2) as tscrp, \
             tc.tile_pool(name="ps", bufs=6, space="PSUM") as psp, \
             tc.tile_pool(name="pso", bufs=1, space="PSUM") as psop:

            init_t = cpool.tile([128, 256], F32)
            rall_t = cpool.tile([128, 5120], F32)
            gd_t = cpool.tile([128, 24 * 128], F32)
            fd_t = cpool.tile([128, 24 * 256], F32)
            qm_t = cpool.tile([128, 256], F32)
            id_t = cpool.tile([128, 128], F32)
            misc_t = cpool.tile([128, 2], F32)
            nc.sync.dma_start(init_t[:], init_e[:])
            for g in range(4):
                nc.sync.dma_start(gd_t[32 * g:32 * g + 4, :], gdd_e[4 * g:4 * g + 4, :])
                nc.sync.dma_start(fd_t[32 * g:32 * g + 4, :], fdd_e[4 * g:4 * g + 4, :])
            nc.sync.dma_start(rall_t[:], rall_e[:])
            nc.sync.dma_start(qm_t[:], qm_e[:])
            nc.sync.dma_start(id_t[:], id_e[:])
            nc.sync.dma_start(misc_t[:], misc_e[:])
            ones_t = misc_t[:, 0:1]
            negpi_t = misc_t[:, 1:2]

            SB = [spool.tile([128, BS * 256], F32, name="sb0"),
                  spool.tile([128, BS * 256], F32, name="sb1")]
            acc_t = spool.tile([128, 64], F32)
            out_t = spool.tile([1, 64], F32)
            y_t = spool.tile([1, 32], F32)

            def dstep(layer, src_r, src_i, dst):
                # state *= exp(-i*ang); [cos(ang)|sin(ang)] is rank-4 in
                # (a,c), computed as one K=4 matmul from host trig data
                for n in range(BS):
                    ps = psp.tile([128, 256], F32, name="ps")
                    t = layer * BS + n
                    g, j = t % 4, t // 4
                    nc.tensor.matmul(ps, gd_t[32 * g:32 * g + 4, j * 128:j * 128 + 128],
                                     fd_t[32 * g:32 * g + 4, j * 256:j * 256 + 256],
                                     start=True, stop=True, tile_position=(32 * g, 0))
                    cssn = scrp.tile([128, 256], F32)
                    nc.scalar.copy(cssn, ps)
                    cs = cssn[:, 0:128]
                    sn = cssn[:, 128:256]
                    sr, si = src_r(n), src_i(n)
                    dr = dst[:, n * 256:n * 256 + 128]
                    di = dst[:, n * 256 + 128:(n + 1) * 256]
                    t1 = scrp.tile([128, 128], F32)
                    nc.gpsimd.tensor_tensor(t1, sr, cs, ALU.mult)
                    t2 = scrp.tile([128, 128], F32)
                    nc.gpsimd.tensor_tensor(t2, si, sn, ALU.mult)
                    nc.vector.tensor_tensor(dr, t1, t2, ALU.add)
                    t3 = scrp.tile([128, 128], F32)
                    nc.vector.tensor_tensor(t3, si, cs, ALU.mult)
                    t4 = scrp.tile([128, 128], F32)
                    nc.gpsimd.tensor_tensor(t4, sr, sn, ALU.mult)
                    nc.vector.tensor_tensor(di, t3, t4, ALU.subtract)

            def rstep(s, src, dst):
                base = s * 512
                for n in range(BS):
                    ps = psp.tile([128, 256], F32, name="ps")
                    sr = src[:, n * 256:n * 256 + 128]
                    si = src[:, n * 256 + 128:(n + 1) * 256]
                    nc.tensor.matmul(ps, sr, rall_t[:, base:base + 256],
                                     start=True, stop=False)
                    nc.tensor.matmul(ps, si, rall_t[:, base + 256:base + 512],
                                     start=False, stop=True)
                    nc.scalar.copy(dst[:, n * 256:(n + 1) * 256], ps)

            # step sequence: D0, R0, R1, R2, R3, D1, R4, R5, R6, R7, D2, R8, R9
            dstep(0, lambda n: init_t[:, 0:128], lambda n: init_t[:, 128:256], SB[0])
            cur = 0
            for s in range(4):
                rstep(s, SB[cur], SB[1 - cur]); cur = 1 - cur
            dstep(1, lambda n: SB[cur][:, n * 256:n * 256 + 128],
                  lambda n: SB[cur][:, n * 256 + 128:(n + 1) * 256], SB[1 - cur])
            cur = 1 - cur
            for s in range(4, 8):
                rstep(s, SB[cur], SB[1 - cur]); cur = 1 - cur
            dstep(2, lambda n: SB[cur][:, n * 256:n * 256 + 128],
                  lambda n: SB[cur][:, n * 256 + 128:(n + 1) * 256], SB[1 - cur])
            cur = 1 - cur
            for s in range(8, 10):
                rstep(s, SB[cur], SB[1 - cur]); cur = 1 - cur
            # final L_A state in SB[cur]; transpose to SB[1-cur] (L_B)
            A, Bt = SB[cur], SB[1 - cur]
            for n in range(BS):
                ps = psp.tile([128, 256], F32, name="ps")
                nc.tensor.transpose(ps[:, 0:128], A[:, n * 256:n * 256 + 128], id_t[:])
                nc.tensor.transpose(ps[:, 128:256], A[:, n * 256 + 128:(n + 1) * 256], id_t[:])
                nc.scalar.copy(Bt[:, n * 256:(n + 1) * 256], ps)
            # measurement: quadratic forms with Qh (L_A) and Ql (L_B)
            for n in range(BS):
                sl = slice(n * 256, (n + 1) * 256)
                ps = psp.tile([128, 256], F32, name="ps")
                nc.tensor.matmul(ps, qm_t[:, 0:128], A[:, sl], start=True, stop=True)
                sc = tscrp.tile([128, 256], F32)
                nc.vector.tensor_tensor(sc, A[:, sl], ps, ALU.mult)
                nc.vector.tensor_reduce(acc_t[:, n:n + 1], sc,
                                        mybir.AxisListType.X, ALU.add)
                ps2 = psp.tile([128, 256], F32, name="ps")
                nc.tensor.matmul(ps2, qm_t[:, 128:256], Bt[:, sl], start=True, stop=True)
                sc2 = tscrp.tile([128, 256], F32)
                nc.vector.tensor_tensor(sc2, Bt[:, sl], ps2, ALU.mult)
                nc.vector.tensor_reduce(acc_t[:, 32 + n:33 + n], sc2,
                                        mybir.AxisListType.X, ALU.add)
            pso = psop.tile([1, 64], F32)
            nc.tensor.matmul(pso, ones_t, acc_t[:], start=True, stop=True)
            nc.scalar.copy(out_t[:], pso)
            nc.vector.tensor_tensor(y_t[:], out_t[:, 0:32], out_t[:, 32:64], ALU.add)
            nc.sync.dma_start(y_e[:], y_t[:])

    _patch_drains(nc)
    _PROG = nc
    return nc


def _rot_mats(angles):
    # angles (..., 3) -> (..., 2, 2) complex128
    phi, theta, omega = angles[..., 0], angles[..., 1], angles[..., 2]
    c = np.cos(theta / 2)
    s = np.sin(theta / 2)
    ep = np.exp(-0.5j * (phi + omega))
    em = np.exp(-0.5j * (phi - omega))
    out = np.empty(angles.shape[:-1] + (2, 2), np.complex128)
    out[..., 0, 0] = ep * c
    out[..., 0, 1] = -np.conj(em) * s
    out[..., 1, 0] = em * s
    out[..., 1, 1] = np.conj(ep) * c
    return out


def _kron7(Us):
    M = Us[0]
    for q in range(1, 7):
        M = np.kron(M, Us[q])
    return M


def _host_prep(x, psq, ptq, penc, pc):
    x = x.astype(np.float64)
    psq = psq.astype(np.float64)
    ptq = ptq.astype(np.float64)
    penc = penc.astype(np.float64)
    pc = pc.astype(np.float64)

    U = _rot_mats(psq)  # (3, 2, 14, 2, 2)
    Mh = {(l, r): _kron7([U[l, r, q] for q in range(7)])
          for l in range(LAYERS) for r in range(2)}
    Ml = {(l, r): _kron7([U[l, r, q] for q in range(7, 14)])
          for l in range(LAYERS) for r in range(2)}

    # initial state after layer-0 round-0 gates on |0...0>
    col = np.outer(Mh[(0, 0)][:, 0], Ml[(0, 0)][:, 0])  # (128,128)
    init = np.concatenate([col.real, col.imag], axis=1).astype(np.float32)

    rall = np.zeros((128, 5120), np.float32)
    for s, (l, r, half) in enumerate(R_STEPS):
        M = Mh[(l, r)] if half == "h" else Ml[(l, r)]
        R1 = np.concatenate([M.real.T, M.imag.T], axis=1)
        R2 = np.concatenate([-M.imag.T, M.real.T], axis=1)
        rall[:, s * 512:s * 512 + 256] = R1
        rall[:, s * 512 + 256:(s + 1) * 512] = R2

    # Z sign tables: qubit q<=6 lives in a (partition) bits, q>=7 in c bits
    a_idx = np.arange(128)
    Za = 1.0 - 2.0 * ((a_idx[None, :] >> (6 - np.arange(7)[:, None])) & 1)   # (7,128) q=0..6
    Zc = 1.0 - 2.0 * ((a_idx[None, :] >> (13 - np.arange(7, 14)[:, None])) & 1)  # (7,128) q=7..13

    # per-sample diagonal factors
    x2 = x.reshape(B, NQ, 2)
    feat = np.einsum("npd,npd->np", x2[:, :-1], x2[:, 1:])  # (B,13)
    pflat = np.arange(13) * NQ + np.arange(1, 14)
    theta = ptq[:, pflat][:, None, :] + penc[:, pflat][:, None, :] * feat[None, :, :]  # (3,B,13)
    th = theta / 2

    ZZa = Za[:6] * Za[1:7]      # (6,128): pairs p=0..5
    ZZc = Zc[:6] * Zc[1:7]      # (6,128): pairs p=7..12 (indexed 0..5)
    H = np.einsum("lnp,pa->lna", th[:, :, 0:6], ZZa)    # (3,B,128)
    L = np.einsum("lnp,pc->lnc", th[:, :, 7:13], ZZc)   # (3,B,128)
    t6 = th[:, :, 6]  # (3,B)

    # measurement matrices
    Qh = np.diag((pc[0:7, None] * Za).sum(0))
    Ql = np.diag((pc[7:14, None] * Zc).sum(0))
    for q in range(7):
        Qh[a_idx, a_idx ^ (1 << (6 - q))] += pc[13 - q]
    for q in range(7, 14):
        Ql[a_idx, a_idx ^ (1 << (13 - q))] += pc[13 - q]
    qm = np.concatenate([Qh, Ql], axis=1).astype(np.float32)

    ident = np.eye(128, dtype=np.float32)
    misc = np.stack([np.ones(128), np.full(128, -np.pi)], axis=1).astype(np.float32)

    in_maps = []
    for k in range(NCORES):
        gdd = np.zeros((16, 24 * 128), np.float32)
        fdd = np.zeros((16, 24 * 256), np.float32)
        for l in range(LAYERS):
            for n in range(BS):
                nb = k * BS + n
                t = l * BS + n
                g, j = t % 4, t // 4
                gs = slice(j * 128, (j + 1) * 128)
                fc = slice(j * 256, j * 256 + 128)
                fs = slice(j * 256 + 128, (j + 1) * 256)
                cosA, sinA = np.cos(H[l, nb]), np.sin(H[l, nb])
                cosB, sinB = np.cos(L[l, nb]), np.sin(L[l, nb])
                c6, s6 = np.cos(t6[l, nb]), np.sin(t6[l, nb])
                za, zc = Za[6], Zc[0]
                gdd[4 * g + 0, gs] = cosA
                gdd[4 * g + 1, gs] = sinA
                gdd[4 * g + 2, gs] = za * cosA
                gdd[4 * g + 3, gs] = za * sinA
                fdd[4 * g + 0, fc] = c6 * cosB
                fdd[4 * g + 1, fc] = -c6 * sinB
                fdd[4 * g + 2, fc] = -s6 * zc * sinB
                fdd[4 * g + 3, fc] = -s6 * zc * cosB
                fdd[4 * g + 0, fs] = c6 * sinB
                fdd[4 * g + 1, fs] = c6 * cosB
                fdd[4 * g + 2, fs] = s6 * zc * cosB
                fdd[4 * g + 3, fs] = -s6 * zc * sinB
        in_maps.append({
            "init": init,
            "rall": rall,
            "gdd": gdd,
            "fdd": fdd,
            "qm": qm,
            "ident": ident,
            "misc": misc,
        })
    return in_maps


_LAST_RES = None


def kernel(x, param_single_qubit, param_two_qubit, param_encoding, param_classical,
           _trace=False):
    global _LAST_RES
    nc = _build()
    in_maps = _host_prep(np.asarray(x), np.asarray(param_single_qubit),
                         np.asarray(param_two_qubit), np.asarray(param_encoding),
                         np.asarray(param_classical))
    kw = {"trace": True} if _trace else {}
    res = run_bass_kernel_spmd(nc, in_maps, list(range(NCORES)), **kw)
    _LAST_RES = res
    out = np.concatenate([res.results[k]["y"].reshape(BS) for k in range(NCORES)])
    return out.astype(np.float32)
